# revision 4
# baseline (speedup 1.0000x reference)
"""AttentionBlock (GroupNorm + single-head self-attention + residual) on 8 TRN2
NeuronCores, data-parallel over the batch dim (B=8, one batch element per core).

Per-core computation (C=512 channels, N=H*W=4096 pixels), v2 design:
  stats: per-group mean/var of x -> per-channel a, b (a = gs*rstd, b = gb-mu*a);
         rstd computed as exp(-0.5*ln(var+eps)) so the whole kernel stays inside
         the single natural_log_exp ACT table set (no per-iteration reloads).
  x_a = a*x cast to fp8 pairs ONCE (replaces the double h-cast of the v1 design);
  the GroupNorm shift b is folded algebraically:
    - k-side shift cancels exactly in softmax (constant per q-column),
    - q-side shift -> bq_eff = bq + Wq b   (device matvec, x64-fp8-rhs trick),
    - v-side shift -> out bias += Wo Wv b  (device matvec chain).
  k = Wk x_a (no bias), q = Wq x_a + bq_eff, vT = x_a^T Wv^T.
  S^T = k^T q  (scores transposed, [k_pix, q_pix]); P^T = exp(S^T/sqrt(C)).
  denominator: DVE/GPSIMD accumulation of P^T tiles + one f32 ones-matmul that
  both partition-reduces and broadcasts -> rb = 1/denom [128, q] (off the PE's
  critical path; frees a PSUM bank and ~4us/chunk of PE time vs the v1 design).
  O_norm = (v P^T) * rb applied in the PSUM->fp8 cast (no extra pass);
  out = x + bo_eff + Wo O_norm.
PSUM: 4-bank shared ring (scores/q-proj/out-proj/broadcast) + 4 banks for the
O accumulators. Out-stores ride the ACT HWDGE queue one chunk delayed so the SP
queue stays free for the next iteration's x loads (cross-iteration pipelining).
"""

import numpy as np
import ml_dtypes
from contextlib import ExitStack

import concourse.bass as bass
import concourse.tile as tile
from concourse import bacc, mybir
from concourse.bass_utils import run_bass_kernel_spmd

C = 512
GROUPS = 32
EPS = 1e-6
CT = C // 128          # 4 channel tiles of 128
CHUNK = 512            # q-chunk width (one PSUM bank of fp32)
F32 = mybir.dt.float32
BF16 = mybir.dt.bfloat16
FP8 = mybir.dt.float8e4
DR = mybir.MatmulPerfMode.DoubleRow
AF = mybir.ActivationFunctionType
ALU = mybir.AluOpType
AX = mybir.AxisListType

GPC = C // GROUPS      # channels per group = 16
GPT = 128 // GPC       # groups per channel-tile = 8
B64 = 64.0             # fp8 staging scale for the tiny bias matvecs


def build_nc(n_pix=4096, repeat=1):
    """repeat>1 wraps the whole body in a hardware loop — used only for timing
    (amortizes the ~80ms per-call axon dispatch overhead over R executions)."""
    nt = n_pix // 128          # number of 128-wide pixel tiles (k tiles)
    nchunk = n_pix // CHUNK    # number of q chunks
    inv_cnt = 1.0 / (GPC * n_pix)
    scale_s = 1.0 / float(np.sqrt(C))

    nc = bacc.Bacc(trn_type="TRN2", target_bir_lowering=False, debug=False)

    xd = nc.declare_dram_parameter("x", [C, n_pix], F32, isOutput=False)
    wqd = nc.declare_dram_parameter("wqT2", [CT // 2, 128, 2, C], FP8, isOutput=False)
    wkd = nc.declare_dram_parameter("wkT2", [CT // 2, 128, 2, C], FP8, isOutput=False)
    wvd = nc.declare_dram_parameter("wvT2", [CT // 2, 128, 2, C], FP8, isOutput=False)
    wod = nc.declare_dram_parameter("woT2", [CT // 2, 128, 2, C], FP8, isOutput=False)
    # per-channel vectors packed [128, CT]: column ct = channels ct*128..+128
    gsd = nc.declare_dram_parameter("gn_scale", [128, CT], F32, isOutput=False)
    gbd = nc.declare_dram_parameter("gn_bias", [128, CT], F32, isOutput=False)
    bqd = nc.declare_dram_parameter("bq", [128, CT], F32, isOutput=False)
    bod = nc.declare_dram_parameter("bo", [128, CT], F32, isOutput=False)
    outd = nc.declare_dram_parameter("out", [C, n_pix], F32, isOutput=True)

    gmat_np = np.zeros((128, GPT), np.float32)
    for p in range(128):
        gmat_np[p, p // GPC] = 1.0
    gmat_d = nc.inline_tensor(gmat_np, name="gmat")
    gmat_t_d = nc.inline_tensor(np.ascontiguousarray(gmat_np.T), name="gmat_t")
    ones128_d = nc.inline_tensor(np.ones((128, 128), np.float32), name="ones128")

    with tile.TileContext(nc) as tc, ExitStack() as ctx:
        cp = ctx.enter_context(tc.tile_pool(name="consts", bufs=1))
        res = ctx.enter_context(tc.tile_pool(name="res", bufs=1))
        xload = ctx.enter_context(tc.tile_pool(name="xload", bufs=8))
        scr = ctx.enter_context(tc.tile_pool(name="scr", bufs=2))
        qp = ctx.enter_context(tc.tile_pool(name="qp", bufs=4))
        ptp = ctx.enter_context(tc.tile_pool(name="ptp", bufs=8))
        oup = ctx.enter_context(tc.tile_pool(name="oup", bufs=4))
        rbp = ctx.enter_context(tc.tile_pool(name="rbp", bufs=2))
        dnp = ctx.enter_context(tc.tile_pool(name="dnp", bufs=4))
        ep = ctx.enter_context(tc.tile_pool(name="ep", bufs=12))
        psS = ctx.enter_context(tc.tile_pool(name="psS", bufs=4, space="PSUM"))
        psO = ctx.enter_context(tc.tile_pool(name="psO", bufs=4, space="PSUM"))

        # tiny pre-loop activation forces the exp_and_others table load into
        # the preheader so the loop body needs no per-iteration reload
        warm = cp.tile([1, 1], F32, name="warm", tag="warm")
        nc.vector.memset(warm[:], 1.0)
        nc.scalar.activation(warm[:], warm[:], AF.Square)

        if repeat > 1:
            loop_cm = tc.For_i(0, repeat, hint_engines=(
                mybir.EngineType.PE, mybir.EngineType.Activation,
                mybir.EngineType.DVE, mybir.EngineType.SP,
                mybir.EngineType.Pool))
            loop_cm.__enter__()

        # ---- stats-critical vectors first: the x DMAs must head the ring ----
        def load_vec(dram, label):
            t = cp.tile([128, CT], F32, name=label, tag=label)
            nc.sync.dma_start(t[:], dram.ap())
            return t

        gs_all = load_vec(gsd, "gs_all")
        gb_all = load_vec(gbd, "gb_all")
        gmat = cp.tile([128, GPT], F32, name="gmat_sb", tag="gmat")
        nc.sync.dma_start(gmat[:], gmat_d.ap())
        gmat_t = cp.tile([GPT, 128], F32, name="gmatT_sb", tag="gmatT")
        nc.sync.dma_start(gmat_t[:], gmat_t_d.ap())

        # ---- resident tensors ----
        x_bf = [res.tile([128, n_pix], BF16, name=f"x_bf{ct}", tag=f"x_bf{ct}")
                for ct in range(CT)]
        x_a = [res.tile([128, 2, n_pix], FP8, name=f"x_a{p}", tag=f"x_a{p}")
               for p in range(CT // 2)]
        k2 = [res.tile([128, 2, n_pix], FP8, name=f"k2_{p}", tag=f"k2_{p}")
              for p in range(CT // 2)]
        vT2 = [res.tile([128, 2, C], FP8, name=f"vT2_{i}", tag=f"vT2_{i}")
               for i in range(nt // 2)]

        # ---- phase 1: load x; per-chunk sums / sum-squares / bf16 copy ----
        s_cols = [cp.tile([128, nchunk], F32, name=f"s_cols{ct}", tag=f"s_cols{ct}")
                  for ct in range(CT)]
        ss_cols = [cp.tile([128, nchunk], F32, name=f"ss_cols{ct}", tag=f"ss_cols{ct}")
                   for ct in range(CT)]
        for ct in range(CT):
            rows = slice(ct * 128, (ct + 1) * 128)
            for j in range(nchunk):
                cols = slice(j * CHUNK, (j + 1) * CHUNK)
                xs = xload.tile([128, CHUNK], F32, name=f"xs{ct}_{j}", tag="xs")
                nc.sync.dma_start(xs[:], xd.ap()[rows, cols])
                nc.vector.reduce_sum(s_cols[ct][:, j:j + 1], xs[:], axis=AX.X)
                sq = scr.tile([128, CHUNK], F32, name=f"sq{ct}_{j}", tag="sq")
                nc.scalar.activation(sq[:], xs[:], AF.Square,
                                     accum_out=ss_cols[ct][:, j:j + 1])
                nc.gpsimd.tensor_copy(x_bf[ct][:, cols], xs[:])

        # remaining constants/vectors (not stats-critical)
        ones128 = res.tile([128, 128], F32, name="ones128_sb", tag="ones128")
        nc.sync.dma_start(ones128[:], ones128_d.ap())
        bq_v = load_vec(bqd, "bq_v")
        bo_v = load_vec(bod, "bo_v")

        def load_w(dram, label):
            ws = []
            for p in range(CT // 2):
                t = res.tile([128, 2, C], FP8, name=f"{label}{p}", tag=f"{label}{p}")
                nc.sync.dma_start(t[:], dram.ap()[p])
                ws.append(t)
            return ws

        # weights loaded after x so the stats-critical x DMAs go first on the ring
        wk_f8 = load_w(wkd, "wk")
        wv_f8 = load_w(wvd, "wv")
        wq_f8 = load_w(wqd, "wq")
        wo_f8 = load_w(wod, "wo")

        stats_all = cp.tile([128, 2 * CT], F32, name="stats_all", tag="stats_all")
        for ct in range(CT):
            nc.vector.reduce_sum(stats_all[:, 2 * ct:2 * ct + 1], s_cols[ct][:],
                                 axis=AX.X)
            nc.vector.reduce_sum(stats_all[:, 2 * ct + 1:2 * ct + 2], ss_cols[ct][:],
                                 axis=AX.X)

        # one matmul for all cross-partition group sums: [128, 8] -> [8, 8]
        pg = psS.tile([GPT, 2 * CT], F32, name="pg", tag="pa")
        nc.tensor.matmul(pg[:], lhsT=gmat[:], rhs=stats_all[:], start=True, stop=True)
        gsb = cp.tile([GPT, 2 * CT], F32, name="gsb", tag="gsb")
        nc.scalar.copy(gsb[:], pg[:])

        mu44 = cp.tile([GPT, CT], F32, name="mu44", tag="mu44")
        ex2 = cp.tile([GPT, CT], F32, name="ex2", tag="ex2")
        musq = cp.tile([GPT, CT], F32, name="musq", tag="musq")
        var44 = cp.tile([GPT, CT], F32, name="var44", tag="var44")
        vare = cp.tile([GPT, CT], F32, name="vare", tag="vare")
        rstd44 = cp.tile([GPT, CT], F32, name="rstd44", tag="rstd44")
        mr = cp.tile([GPT, 2 * CT], F32, name="mr", tag="mr")
        nc.scalar.mul(mu44[:], gsb[0:GPT, 0:2 * CT:2], inv_cnt)
        nc.scalar.mul(ex2[:], gsb[0:GPT, 1:2 * CT:2], inv_cnt)
        nc.vector.tensor_mul(musq[:], mu44[:], mu44[:])
        nc.vector.tensor_sub(var44[:], ex2[:], musq[:])
        nc.vector.tensor_scalar_add(vare[:], var44[:], EPS)
        # rstd = rsqrt(var+eps) via DVE-only Newton (seed (1+1/v)/2, 3 steps):
        # avoids AF.Sqrt/AF.Ln so the only ACT table set used anywhere is
        # exp_and_others — the per-iteration table reloads of v1 disappear.
        rcpv = cp.tile([GPT, CT], F32, name="rcpv", tag="rcpv")
        nc.vector.reciprocal(rcpv[:], vare[:])
        nc.vector.tensor_scalar(rstd44[:], rcpv[:], 1.0, 0.5,
                                op0=ALU.add, op1=ALU.mult)
        nwt = cp.tile([GPT, CT], F32, name="nwt", tag="nwt")
        for _ in range(3):
            nc.vector.tensor_mul(nwt[:], rstd44[:], rstd44[:])
            nc.vector.tensor_mul(nwt[:], nwt[:], vare[:])
            nc.vector.tensor_scalar(nwt[:], nwt[:], -0.5, 1.5,
                                    op0=ALU.mult, op1=ALU.add)
            nc.vector.tensor_mul(rstd44[:], rstd44[:], nwt[:])
        nc.vector.tensor_copy(mr[0:GPT, 0:2 * CT:2], mu44[:])
        nc.vector.tensor_copy(mr[0:GPT, 1:2 * CT:2], rstd44[:])

        # broadcast group mu/rstd back to channels: [8, 8] -> [128, 8]
        pmc = psS.tile([128, 2 * CT], F32, name="pmc", tag="pa")
        nc.tensor.matmul(pmc[:], lhsT=gmat_t[:], rhs=mr[:], start=True, stop=True)
        mcall = cp.tile([128, 2 * CT], F32, name="mcall", tag="mcall")
        nc.scalar.copy(mcall[:], pmc[:])
        a_all = cp.tile([128, CT], F32, name="a_all", tag="a_all")
        nc.vector.tensor_mul(a_all[:], mcall[:, 1:2 * CT:2], gs_all[:])
        btmp = cp.tile([128, CT], F32, name="btmp", tag="btmp")
        nc.vector.tensor_mul(btmp[:], mcall[:, 0:2 * CT:2], a_all[:])
        b_all = cp.tile([128, CT], F32, name="b_all", tag="b_all")
        nc.vector.tensor_sub(b_all[:], gb_all[:], btmp[:])

        # ---- GroupNorm-shift folding: tiny matvecs via x64 fp8 staging ----
        # b64 = 64*b packed as DoubleRow rhs [128, 2, 1] per weight tile
        b64 = [cp.tile([128, 2, 1], FP8, name=f"b64_{t}", tag=f"b64_{t}")
               for t in range(CT // 2)]
        for t in range(CT // 2):
            for r in range(2):
                nc.scalar.mul(b64[t][:, r, 0:1], b_all[:, 2 * t + r:2 * t + r + 1],
                              B64)

        def matvec(ws, rhs, tag):
            """[128, CT] psum: col m = (W.T2-packed W) @ rhs (rhs fp8 pairs)."""
            pv = psS.tile([128, CT], F32, name=f"mv_{tag}", tag="pa")
            for m in range(CT):
                for t in range(CT // 2):
                    nc.tensor.matmul(pv[:, m:m + 1],
                                     lhsT=ws[t][:, :, m * 128:(m + 1) * 128],
                                     rhs=rhs[t][:],
                                     start=(t == 0), stop=(t == CT // 2 - 1),
                                     perf_mode=DR)
            return pv

        pqb = matvec(wq_f8, b64, "qb")         # 64 * Wq b
        bq_comb = cp.tile([128, CT], F32, name="bq_comb", tag="bq_comb")
        nc.vector.scalar_tensor_tensor(bq_comb[:], pqb[:], 1.0 / B64, bq_v[:],
                                       op0=ALU.mult, op1=ALU.add)
        pvb = matvec(wv_f8, b64, "vb")         # 64 * Wv b
        wvb64 = [cp.tile([128, 2, 1], FP8, name=f"wvb_{t}", tag=f"wvb_{t}")
                 for t in range(CT // 2)]
        for t in range(CT // 2):
            for r in range(2):
                nc.scalar.copy(wvb64[t][:, r, 0:1],
                               pvb[:, 2 * t + r:2 * t + r + 1])
        pob = matvec(wo_f8, wvb64, "ob")       # 64 * Wo Wv b
        bo_comb = cp.tile([128, CT], F32, name="bo_comb", tag="bo_comb")
        nc.vector.scalar_tensor_tensor(bo_comb[:], pob[:], 1.0 / B64, bo_v[:],
                                       op0=ALU.mult, op1=ALU.add)

        # ---- phase 2: x_a = a*x fp8 cast; k and vT projections ----
        for j in range(nchunk):
            cols = slice(j * CHUNK, (j + 1) * CHUNK)
            for ct in range(CT):
                adst = x_a[ct // 2][:, ct % 2, cols]
                acol = a_all[:, ct:ct + 1]
                if ct % 2 == 0:
                    nc.gpsimd.tensor_scalar(adst, x_bf[ct][:, cols], acol, None,
                                            op0=ALU.mult)
                else:
                    nc.vector.tensor_scalar(adst, x_bf[ct][:, cols], acol, None,
                                            op0=ALU.mult)
            for ct in range(CT):
                pk = psS.tile([128, CHUNK], F32, name=f"pk{ct}_{j}", tag="pa")
                for t in range(CT // 2):
                    nc.tensor.matmul(pk[:],
                                     lhsT=wk_f8[t][:, :, ct * 128:(ct + 1) * 128],
                                     rhs=x_a[t][:, :, cols],
                                     start=(t == 0), stop=(t == CT // 2 - 1),
                                     perf_mode=DR)
                kdst = k2[ct // 2][:, ct % 2, cols]
                if ct % 2 == 0:
                    nc.scalar.copy(kdst, pk[:])
                else:
                    nc.vector.tensor_copy(kdst, pk[:])
            for i in range(4 * j, 4 * j + 4):
                off = (i - 4 * j) * 128
                pv = psS.tile([128, C], F32, name=f"pv{i}", tag="pa")
                for t in range(CT // 2):
                    nc.tensor.matmul(pv[:],
                                     lhsT=x_a[t][:, :, j * CHUNK + off:
                                                 j * CHUNK + off + 128],
                                     rhs=wv_f8[t][:],
                                     start=(t == 0), stop=(t == CT // 2 - 1),
                                     perf_mode=DR)
                vdst = vT2[i // 2][:, i % 2, :]
                if i % 2 == 0:
                    nc.scalar.copy(vdst, pv[:])
                else:
                    nc.vector.tensor_copy(vdst, pv[:])

        # ---- phase 3: attention, one q-chunk at a time ----
        def q_proj(ch):
            cols = slice(ch * CHUNK, (ch + 1) * CHUNK)
            qs = [qp.tile([128, 2, CHUNK], FP8, name=f"qs{ch}_{p}", tag="qs")
                  for p in range(CT // 2)]
            for m in range(CT):
                pq = psS.tile([128, CHUNK], F32, name=f"pq{ch}_{m}", tag="pa")
                for t in range(CT // 2):
                    nc.tensor.matmul(pq[:],
                                     lhsT=wq_f8[t][:, :, m * 128:(m + 1) * 128],
                                     rhs=x_a[t][:, :, cols],
                                     start=(t == 0), stop=(t == CT // 2 - 1),
                                     perf_mode=DR)
                qdst = qs[m // 2][:, m % 2, :]
                if m % 2 == 0:
                    nc.scalar.activation(qdst, pq[:], AF.Identity,
                                         bias=bq_comb[:, m:m + 1])
                else:
                    nc.vector.tensor_scalar(qdst, pq[:], bq_comb[:, m:m + 1],
                                            None, op0=ALU.add)
            return qs

        qs = q_proj(0)
        pending_stores = []
        for ch in range(nchunk):
            cols = slice(ch * CHUNK, (ch + 1) * CHUNK)
            po = [psO.tile([128, CHUNK], F32, name=f"po{ch}_{ct}", tag="po")
                  for ct in range(CT)]
            npair = nt // 2
            pts = [None] * npair
            # per-chunk softmax-denominator accumulators (DVE + GPSIMD halves)
            dnD = dnp.tile([128, CHUNK], F32, name=f"dnD{ch}", tag="dnD")
            dnG = dnp.tile([128, CHUNK], F32, name=f"dnG{ch}", tag="dnG")

            def o_pair(pp):
                for ct in range(CT):
                    nc.tensor.matmul(po[ct][:],
                                     lhsT=vT2[pp][:, :, ct * 128:(ct + 1) * 128],
                                     rhs=pts[pp][:],
                                     start=(pp == 0), stop=(pp == npair - 1),
                                     perf_mode=DR)

            for kt in range(nt):
                ps = psS.tile([128, CHUNK], F32, name=f"ps{ch}_{kt}", tag="pa")
                for t in range(CT // 2):
                    nc.tensor.matmul(ps[:],
                                     lhsT=k2[t][:, :, kt * 128:(kt + 1) * 128],
                                     rhs=qs[t][:],
                                     start=(t == 0), stop=(t == CT // 2 - 1),
                                     perf_mode=DR)
                if kt % 2 == 0:
                    pts[kt // 2] = ptp.tile([128, 2, CHUNK], FP8,
                                            name=f"pt{ch}_{kt}", tag="pt")
                pt_half = pts[kt // 2][:, kt % 2, :]
                nc.scalar.activation(pt_half, ps[:], AF.Exp, scale=scale_s)
                if kt % 2 == 1:
                    pp = kt // 2
                    eng, dn = (nc.vector, dnD) if pp % 2 == 0 else (nc.gpsimd, dnG)
                    pt0, pt1 = pts[pp][:, 0, :], pts[pp][:, 1, :]
                    if pp < 2:
                        eng.tensor_add(dn[:], pt0, pt1)
                    else:
                        eng.tensor_add(dn[:], dn[:], pt0)
                        eng.tensor_add(dn[:], dn[:], pt1)
                    # O matmuls lag one completed pair (keeps PE off the ACT path)
                    if kt >= 3:
                        o_pair(pp - 1)
                # previous chunk's out-stores: waits long satisfied by now
                if kt == 4 and pending_stores:
                    for ap_out, osb_t in pending_stores:
                        nc.scalar.dma_start(ap_out, osb_t[:])
                    pending_stores = []
                # next chunk's q projection fills PE slack from ring backpressure
                if kt == 25 and ch + 1 < nchunk:
                    qs_next = q_proj(ch + 1)
            o_pair(npair - 1)

            # denominator: merge halves, one f32 ones-matmul does the
            # cross-partition reduce AND the broadcast to all 128 partitions
            nc.vector.tensor_add(dnD[:], dnD[:], dnG[:])
            pbc = psS.tile([128, CHUNK], F32, name=f"pbc{ch}", tag="pa")
            nc.tensor.matmul(pbc[:], lhsT=ones128[:], rhs=dnD[:],
                             start=True, stop=True)
            rb = rbp.tile([128, CHUNK], F32, name=f"rb{ch}", tag="rb")
            nc.vector.reciprocal(rb[:], pbc[:])

            # normalized O -> fp8 pairs (normalization fused into the cast)
            ou = [oup.tile([128, 2, CHUNK], FP8, name=f"ou{ch}_{p}", tag="ou")
                  for p in range(CT // 2)]
            for ct in range(CT):
                nc.vector.tensor_mul(ou[ct // 2][:, ct % 2, :], po[ct][:], rb[:])

            # output projection + residual + bias
            for oct in range(CT):
                pz = psS.tile([128, CHUNK], F32, name=f"pz{ch}_{oct}", tag="pa")
                for t in range(CT // 2):
                    nc.tensor.matmul(pz[:],
                                     lhsT=wo_f8[t][:, :, oct * 128:(oct + 1) * 128],
                                     rhs=ou[t][:],
                                     start=(t == 0), stop=(t == CT // 2 - 1),
                                     perf_mode=DR)
                xr = ep.tile([128, CHUNK], F32, name=f"xr{ch}_{oct}", tag="xr")
                nc.sync.dma_start(xr[:], xd.ap()[oct * 128:(oct + 1) * 128, cols])
                osb = ep.tile([128, CHUNK], F32, name=f"osb{ch}_{oct}", tag="osb")
                nc.vector.scalar_tensor_tensor(osb[:], pz[:], bo_comb[:, oct:oct + 1],
                                               xr[:], op0=ALU.add, op1=ALU.add)
                pending_stores.append(
                    (outd.ap()[oct * 128:(oct + 1) * 128, cols], osb))

            if ch + 1 < nchunk:
                qs = qs_next

        for ap_out, osb_t in pending_stores:
            nc.scalar.dma_start(ap_out, osb_t[:])

        if repeat > 1:
            loop_cm.__exit__(None, None, None)

    nc.compile()
    return nc


_NC_CACHE = {}


def _get_nc(n_pix):
    if n_pix not in _NC_CACHE:
        _NC_CACHE[n_pix] = build_nc(n_pix)
    return _NC_CACHE[n_pix]


def make_in_maps(x, gn_scale, gn_bias, Wq, bq, Wk, bk, Wv, bv, Wo, bo):
    B, C_, H, W = x.shape
    n_pix = H * W

    def vec(v):
        return np.ascontiguousarray(
            np.asarray(v, np.float32).reshape(CT, 128).T)

    def wT2(w):
        """wT [C, C] -> pair-packed [CT//2, 128, 2, C] fp8 (DoubleRow layout)."""
        wt = np.asarray(w, np.float32).T.reshape(CT // 2, 2, 128, C)
        return np.ascontiguousarray(
            wt.transpose(0, 2, 1, 3).astype(ml_dtypes.float8_e4m3))

    # v-bias folds into the output bias: softmax rows sum to 1, so
    # out = x + Wo @ (v_0 P^T / denom) + (bo + Wo @ bv); the GroupNorm-shift
    # part of the v/q biases is folded on-device (see build_nc).
    bo_eff = (np.asarray(bo, np.float64)
              + np.asarray(Wo, np.float64) @ np.asarray(bv, np.float64))
    base = {
        "wqT2": wT2(Wq),
        "wkT2": wT2(Wk),
        "wvT2": wT2(Wv),
        "woT2": wT2(Wo),
        "gn_scale": vec(gn_scale),
        "gn_bias": vec(gn_bias),
        "bq": vec(bq),
        "bo": vec(bo_eff),
    }
    f32 = lambda v: np.ascontiguousarray(np.asarray(v, np.float32))
    return [dict(base, x=f32(np.asarray(x[b], np.float32).reshape(C_, n_pix)))
            for b in range(B)]


def kernel(x, gn_scale, gn_bias, Wq, bq, Wk, bk, Wv, bv, Wo, bo):
    x = np.asarray(x)
    B, C_, H, W = x.shape
    n_pix = H * W
    nc = _get_nc(n_pix)
    in_maps = make_in_maps(x, gn_scale, gn_bias, Wq, bq, Wk, bk, Wv, bv, Wo, bo)
    res = run_bass_kernel_spmd(nc, in_maps, core_ids=list(range(B)))
    out = np.stack([res.results[b]["out"] for b in range(B)])
    return out.reshape(B, C_, H, W).astype(np.float32)


# revision 5
# speedup vs baseline: 1.2319x; 1.2319x over previous
"""AttentionBlock (GroupNorm + single-head self-attention + residual) on 8 TRN2
NeuronCores, data-parallel over the batch dim (B=8, one batch element per core).

Per-core computation (C=512 channels, N=H*W=4096 pixels), v3 design:
  stats: per-group mean/var of x -> per-channel a, b (a = gs*rstd, b = gb-mu*a);
         rstd via DVE-only Newton so the only ACT table set used anywhere is
         exp_and_others (no per-iteration table reloads).
  x is staged DIRECTLY as fp8 pairs at load time (one cast per chunk, off the
  stats critical path); the GroupNorm scale a folds into Wq/Wk/Wv on device
  (bf16-staged weights, 12 scale ops once per call); the shift b folds
  algebraically:
    - k-side shift cancels exactly in softmax (constant per q-column),
    - q-side shift -> bq_eff = bq + Wq' (b/a)   (tiny matvec, x64-fp8 trick),
    - v-side shift -> out bias += Wo Wv' (b/a)  (tiny matvec chain).
  k = Wk' x_f8 (no bias), q = Wq' x_f8 + bq_eff, vT = x_f8^T Wv'^T.
  S^T = k^T q; P^T = exp(S^T/sqrt(C)).
  denominator: two interleaved DVE accumulation chains over the P^T tiles +
  one f32 ones-matmul that both partition-reduces and broadcasts ->
  rb = 1/denom [128, q]; normalization fused into the O PSUM->fp8 cast.
  out = x + bo_eff + Wo O_norm.
PSUM: 4-bank shared ring (scores/q-proj/out-proj/broadcast) + 4 O banks.
"""

import numpy as np
import ml_dtypes
from contextlib import ExitStack

import concourse.bass as bass
import concourse.tile as tile
from concourse import bacc, mybir
from concourse.bass_utils import run_bass_kernel_spmd

C = 512
GROUPS = 32
EPS = 1e-6
CT = C // 128          # 4 channel tiles of 128
CHUNK = 512            # q-chunk width (one PSUM bank of fp32)
F32 = mybir.dt.float32
BF16 = mybir.dt.bfloat16
FP8 = mybir.dt.float8e4
DR = mybir.MatmulPerfMode.DoubleRow
AF = mybir.ActivationFunctionType
ALU = mybir.AluOpType
AX = mybir.AxisListType

GPC = C // GROUPS      # channels per group = 16
GPT = 128 // GPC       # groups per channel-tile = 8
B64 = 64.0             # fp8 staging scale for the tiny bias matvecs


def build_nc(n_pix=4096, repeat=1):
    """repeat>1 wraps the whole body in a hardware loop — used only for timing
    (amortizes the ~80ms per-call axon dispatch overhead over R executions)."""
    nt = n_pix // 128          # number of 128-wide pixel tiles (k tiles)
    nchunk = n_pix // CHUNK    # number of q chunks
    inv_cnt = 1.0 / (GPC * n_pix)
    scale_s = 1.0 / float(np.sqrt(C))

    nc = bacc.Bacc(trn_type="TRN2", target_bir_lowering=False, debug=False)

    xd = nc.declare_dram_parameter("x", [C, n_pix], F32, isOutput=False)
    # q/k/v weights come in bf16 (scaled by the GroupNorm a on device); Wo in fp8
    wqd = nc.declare_dram_parameter("wqT2", [CT // 2, 128, 2, C], BF16, isOutput=False)
    wkd = nc.declare_dram_parameter("wkT2", [CT // 2, 128, 2, C], BF16, isOutput=False)
    wvd = nc.declare_dram_parameter("wvT2", [CT // 2, 128, 2, C], BF16, isOutput=False)
    wod = nc.declare_dram_parameter("woT2", [CT // 2, 128, 2, C], FP8, isOutput=False)
    # per-channel vectors packed [128, CT]: column ct = channels ct*128..+128
    gsd = nc.declare_dram_parameter("gn_scale", [128, CT], F32, isOutput=False)
    gbd = nc.declare_dram_parameter("gn_bias", [128, CT], F32, isOutput=False)
    bqd = nc.declare_dram_parameter("bq", [128, CT], F32, isOutput=False)
    bod = nc.declare_dram_parameter("bo", [128, CT], F32, isOutput=False)
    outd = nc.declare_dram_parameter("out", [C, n_pix], F32, isOutput=True)

    gmat_np = np.zeros((128, GPT), np.float32)
    for p in range(128):
        gmat_np[p, p // GPC] = 1.0
    gmat_d = nc.inline_tensor(gmat_np, name="gmat")
    gmat_t_d = nc.inline_tensor(np.ascontiguousarray(gmat_np.T), name="gmat_t")
    ones128_d = nc.inline_tensor(np.ones((128, 128), np.float32), name="ones128")

    with tile.TileContext(nc) as tc, ExitStack() as ctx:
        cp = ctx.enter_context(tc.tile_pool(name="consts", bufs=1))
        res = ctx.enter_context(tc.tile_pool(name="res", bufs=1))
        xload = ctx.enter_context(tc.tile_pool(name="xload", bufs=8))
        scr = ctx.enter_context(tc.tile_pool(name="scr", bufs=2))
        qp = ctx.enter_context(tc.tile_pool(name="qp", bufs=4))
        ptp = ctx.enter_context(tc.tile_pool(name="ptp", bufs=8))
        oup = ctx.enter_context(tc.tile_pool(name="oup", bufs=4))
        rbp = ctx.enter_context(tc.tile_pool(name="rbp", bufs=2))
        dnp = ctx.enter_context(tc.tile_pool(name="dnp", bufs=4))
        ep = ctx.enter_context(tc.tile_pool(name="ep", bufs=12))
        psS = ctx.enter_context(tc.tile_pool(name="psS", bufs=4, space="PSUM"))
        psO = ctx.enter_context(tc.tile_pool(name="psO", bufs=4, space="PSUM"))

        if repeat > 1:
            loop_cm = tc.For_i(0, repeat, hint_engines=(
                mybir.EngineType.PE, mybir.EngineType.Activation,
                mybir.EngineType.DVE, mybir.EngineType.SP,
                mybir.EngineType.Pool))
            loop_cm.__enter__()

        # ---- stats-critical vectors first: the x DMAs must head the ring ----
        def load_vec(dram, label):
            t = cp.tile([128, CT], F32, name=label, tag=label)
            nc.sync.dma_start(t[:], dram.ap())
            return t

        gs_all = load_vec(gsd, "gs_all")
        gb_all = load_vec(gbd, "gb_all")
        gmat = cp.tile([128, GPT], F32, name="gmat_sb", tag="gmat")
        nc.sync.dma_start(gmat[:], gmat_d.ap())
        gmat_t = cp.tile([GPT, 128], F32, name="gmatT_sb", tag="gmatT")
        nc.sync.dma_start(gmat_t[:], gmat_t_d.ap())

        # ---- resident tensors ----
        x_f8 = [res.tile([128, 2, n_pix], FP8, name=f"x_f8_{p}", tag=f"x_f8_{p}")
                for p in range(CT // 2)]
        k2 = [res.tile([128, 2, n_pix], FP8, name=f"k2_{p}", tag=f"k2_{p}")
              for p in range(CT // 2)]
        vT2 = [res.tile([128, 2, C], FP8, name=f"vT2_{i}", tag=f"vT2_{i}")
               for i in range(nt // 2)]

        # ---- phase 1: load x; per-chunk sums / sum-squares / fp8 staging ----
        s_cols = [cp.tile([128, nchunk], F32, name=f"s_cols{ct}", tag=f"s_cols{ct}")
                  for ct in range(CT)]
        ss_cols = [cp.tile([128, nchunk], F32, name=f"ss_cols{ct}", tag=f"ss_cols{ct}")
                   for ct in range(CT)]
        for ct in range(CT):
            rows = slice(ct * 128, (ct + 1) * 128)
            for j in range(nchunk):
                cols = slice(j * CHUNK, (j + 1) * CHUNK)
                xs = xload.tile([128, CHUNK], F32, name=f"xs{ct}_{j}", tag="xs")
                nc.sync.dma_start(xs[:], xd.ap()[rows, cols])
                nc.vector.reduce_sum(s_cols[ct][:, j:j + 1], xs[:], axis=AX.X)
                sq = scr.tile([128, CHUNK], F32, name=f"sq{ct}_{j}", tag="sq")
                nc.scalar.activation(sq[:], xs[:], AF.Square,
                                     accum_out=ss_cols[ct][:, j:j + 1])
                nc.gpsimd.tensor_copy(x_f8[ct // 2][:, ct % 2, cols], xs[:])

        # remaining constants/vectors (not stats-critical)
        ones128 = res.tile([128, 128], F32, name="ones128_sb", tag="ones128")
        nc.sync.dma_start(ones128[:], ones128_d.ap())
        bq_v = load_vec(bqd, "bq_v")
        bo_v = load_vec(bod, "bo_v")

        def load_w(dram, label, dt):
            ws = []
            for p in range(CT // 2):
                t = res.tile([128, 2, C], dt, name=f"{label}{p}", tag=f"{label}{p}")
                nc.sync.dma_start(t[:], dram.ap()[p])
                ws.append(t)
            return ws

        # weights loaded after x so the stats-critical x DMAs go first on the ring
        wk_bf = load_w(wkd, "wkb", BF16)
        wv_bf = load_w(wvd, "wvb", BF16)
        wq_bf = load_w(wqd, "wqb", BF16)
        wo_f8 = load_w(wod, "wo", FP8)

        stats_all = cp.tile([128, 2 * CT], F32, name="stats_all", tag="stats_all")
        for ct in range(CT):
            nc.vector.reduce_sum(stats_all[:, 2 * ct:2 * ct + 1], s_cols[ct][:],
                                 axis=AX.X)
            nc.vector.reduce_sum(stats_all[:, 2 * ct + 1:2 * ct + 2], ss_cols[ct][:],
                                 axis=AX.X)

        # one matmul for all cross-partition group sums: [128, 8] -> [8, 8]
        pg = psS.tile([GPT, 2 * CT], F32, name="pg", tag="pa")
        nc.tensor.matmul(pg[:], lhsT=gmat[:], rhs=stats_all[:], start=True, stop=True)
        gsb = cp.tile([GPT, 2 * CT], F32, name="gsb", tag="gsb")
        nc.scalar.copy(gsb[:], pg[:])

        mu44 = cp.tile([GPT, CT], F32, name="mu44", tag="mu44")
        ex2 = cp.tile([GPT, CT], F32, name="ex2", tag="ex2")
        musq = cp.tile([GPT, CT], F32, name="musq", tag="musq")
        var44 = cp.tile([GPT, CT], F32, name="var44", tag="var44")
        vare = cp.tile([GPT, CT], F32, name="vare", tag="vare")
        rstd44 = cp.tile([GPT, CT], F32, name="rstd44", tag="rstd44")
        mr = cp.tile([GPT, 2 * CT], F32, name="mr", tag="mr")
        nc.scalar.mul(mu44[:], gsb[0:GPT, 0:2 * CT:2], inv_cnt)
        nc.scalar.mul(ex2[:], gsb[0:GPT, 1:2 * CT:2], inv_cnt)
        nc.vector.tensor_mul(musq[:], mu44[:], mu44[:])
        nc.vector.tensor_sub(var44[:], ex2[:], musq[:])
        nc.vector.tensor_scalar_add(vare[:], var44[:], EPS)
        # rstd = rsqrt(var+eps) via DVE-only Newton (seed (1+1/v)/2, 3 steps):
        # avoids AF.Sqrt/AF.Ln so only the exp_and_others ACT set is needed.
        rcpv = cp.tile([GPT, CT], F32, name="rcpv", tag="rcpv")
        nc.vector.reciprocal(rcpv[:], vare[:])
        nc.vector.tensor_scalar(rstd44[:], rcpv[:], 1.0, 0.5,
                                op0=ALU.add, op1=ALU.mult)
        nwt = cp.tile([GPT, CT], F32, name="nwt", tag="nwt")
        for _ in range(3):
            nc.vector.tensor_mul(nwt[:], rstd44[:], rstd44[:])
            nc.vector.tensor_mul(nwt[:], nwt[:], vare[:])
            nc.vector.tensor_scalar(nwt[:], nwt[:], -0.5, 1.5,
                                    op0=ALU.mult, op1=ALU.add)
            nc.vector.tensor_mul(rstd44[:], rstd44[:], nwt[:])
        nc.vector.tensor_copy(mr[0:GPT, 0:2 * CT:2], mu44[:])
        nc.vector.tensor_copy(mr[0:GPT, 1:2 * CT:2], rstd44[:])

        # broadcast group mu/rstd back to channels: [8, 8] -> [128, 8]
        pmc = psS.tile([128, 2 * CT], F32, name="pmc", tag="pa")
        nc.tensor.matmul(pmc[:], lhsT=gmat_t[:], rhs=mr[:], start=True, stop=True)
        mcall = cp.tile([128, 2 * CT], F32, name="mcall", tag="mcall")
        nc.scalar.copy(mcall[:], pmc[:])
        a_all = cp.tile([128, CT], F32, name="a_all", tag="a_all")
        nc.vector.tensor_mul(a_all[:], mcall[:, 1:2 * CT:2], gs_all[:])
        btmp = cp.tile([128, CT], F32, name="btmp", tag="btmp")
        nc.vector.tensor_mul(btmp[:], mcall[:, 0:2 * CT:2], a_all[:])
        b_all = cp.tile([128, CT], F32, name="b_all", tag="b_all")
        nc.vector.tensor_sub(b_all[:], gb_all[:], btmp[:])

        # ---- fold a into the q/k/v weights: wX' = wX * a (input-channel axis)
        wq_f8 = [res.tile([128, 2, C], FP8, name=f"wq{p}", tag=f"wq{p}")
                 for p in range(CT // 2)]
        wk_f8 = [res.tile([128, 2, C], FP8, name=f"wk{p}", tag=f"wk{p}")
                 for p in range(CT // 2)]
        wv_f8 = [res.tile([128, 2, C], FP8, name=f"wv{p}", tag=f"wv{p}")
                 for p in range(CT // 2)]
        for (wbf, wf8) in ((wq_bf, wq_f8), (wk_bf, wk_f8), (wv_bf, wv_f8)):
            for t in range(CT // 2):
                for r in range(2):
                    acol = a_all[:, 2 * t + r:2 * t + r + 1]
                    if r == 0:
                        nc.scalar.activation(wf8[t][:, r, :], wbf[t][:, r, :],
                                             AF.Identity, scale=acol)
                    else:
                        nc.vector.tensor_scalar(wf8[t][:, r, :], wbf[t][:, r, :],
                                                acol, None, op0=ALU.mult)

        # ---- GroupNorm-shift folding: tiny matvecs via x64 fp8 staging ----
        # rhs = 64 * b/a packed as DoubleRow pairs [128, 2, 1] per weight tile
        boa = cp.tile([128, CT], F32, name="boa", tag="boa")
        ra = cp.tile([128, CT], F32, name="ra", tag="ra")
        nc.vector.reciprocal(ra[:], a_all[:])
        nc.vector.tensor_mul(boa[:], b_all[:], ra[:])
        b64 = [cp.tile([128, 2, 1], FP8, name=f"b64_{t}", tag=f"b64_{t}")
               for t in range(CT // 2)]
        for t in range(CT // 2):
            for r in range(2):
                nc.scalar.mul(b64[t][:, r, 0:1], boa[:, 2 * t + r:2 * t + r + 1],
                              B64)

        def matvec(ws, rhs, tag):
            """[128, CT] psum: col m = (T2-packed W) @ rhs (rhs fp8 pairs)."""
            pv = psS.tile([128, CT], F32, name=f"mv_{tag}", tag="pa")
            for m in range(CT):
                for t in range(CT // 2):
                    nc.tensor.matmul(pv[:, m:m + 1],
                                     lhsT=ws[t][:, :, m * 128:(m + 1) * 128],
                                     rhs=rhs[t][:],
                                     start=(t == 0), stop=(t == CT // 2 - 1),
                                     perf_mode=DR)
            return pv

        pqb = matvec(wq_f8, b64, "qb")         # 64 * Wq b
        bq_comb = cp.tile([128, CT], F32, name="bq_comb", tag="bq_comb")
        nc.vector.scalar_tensor_tensor(bq_comb[:], pqb[:], 1.0 / B64, bq_v[:],
                                       op0=ALU.mult, op1=ALU.add)
        pvb = matvec(wv_f8, b64, "vb")         # 64 * Wv b
        wvb64 = [cp.tile([128, 2, 1], FP8, name=f"wvb_{t}", tag=f"wvb_{t}")
                 for t in range(CT // 2)]
        for t in range(CT // 2):
            for r in range(2):
                nc.scalar.copy(wvb64[t][:, r, 0:1],
                               pvb[:, 2 * t + r:2 * t + r + 1])
        pob = matvec(wo_f8, wvb64, "ob")       # 64 * Wo Wv b
        bo_comb = cp.tile([128, CT], F32, name="bo_comb", tag="bo_comb")
        nc.vector.scalar_tensor_tensor(bo_comb[:], pob[:], 1.0 / B64, bo_v[:],
                                       op0=ALU.mult, op1=ALU.add)

        # ---- phase 2: k and vT projections straight off x_f8 ----
        for j in range(nchunk):
            cols = slice(j * CHUNK, (j + 1) * CHUNK)
            for ct in range(CT):
                pk = psS.tile([128, CHUNK], F32, name=f"pk{ct}_{j}", tag="pa")
                for t in range(CT // 2):
                    nc.tensor.matmul(pk[:],
                                     lhsT=wk_f8[t][:, :, ct * 128:(ct + 1) * 128],
                                     rhs=x_f8[t][:, :, cols],
                                     start=(t == 0), stop=(t == CT // 2 - 1),
                                     perf_mode=DR)
                kdst = k2[ct // 2][:, ct % 2, cols]
                if ct % 2 == 0:
                    nc.scalar.copy(kdst, pk[:])
                else:
                    nc.vector.tensor_copy(kdst, pk[:])
            for i in range(4 * j, 4 * j + 4):
                off = (i - 4 * j) * 128
                pv = psS.tile([128, C], F32, name=f"pv{i}", tag="pa")
                for t in range(CT // 2):
                    nc.tensor.matmul(pv[:],
                                     lhsT=x_f8[t][:, :, j * CHUNK + off:
                                                 j * CHUNK + off + 128],
                                     rhs=wv_f8[t][:],
                                     start=(t == 0), stop=(t == CT // 2 - 1),
                                     perf_mode=DR)
                vdst = vT2[i // 2][:, i % 2, :]
                if i % 2 == 0:
                    nc.scalar.copy(vdst, pv[:])
                else:
                    nc.vector.tensor_copy(vdst, pv[:])

        # ---- phase 3: attention, one q-chunk at a time ----
        def q_proj(ch):
            cols = slice(ch * CHUNK, (ch + 1) * CHUNK)
            qs = [qp.tile([128, 2, CHUNK], FP8, name=f"qs{ch}_{p}", tag="qs")
                  for p in range(CT // 2)]
            for m in range(CT):
                pq = psS.tile([128, CHUNK], F32, name=f"pq{ch}_{m}", tag="pa")
                for t in range(CT // 2):
                    nc.tensor.matmul(pq[:],
                                     lhsT=wq_f8[t][:, :, m * 128:(m + 1) * 128],
                                     rhs=x_f8[t][:, :, cols],
                                     start=(t == 0), stop=(t == CT // 2 - 1),
                                     perf_mode=DR)
                qdst = qs[m // 2][:, m % 2, :]
                if m % 2 == 0:
                    nc.scalar.activation(qdst, pq[:], AF.Identity,
                                         bias=bq_comb[:, m:m + 1])
                else:
                    nc.vector.tensor_scalar(qdst, pq[:], bq_comb[:, m:m + 1],
                                            None, op0=ALU.add)
            return qs

        qs = q_proj(0)
        pending_stores = []
        for ch in range(nchunk):
            cols = slice(ch * CHUNK, (ch + 1) * CHUNK)
            po = [psO.tile([128, CHUNK], F32, name=f"po{ch}_{ct}", tag="po")
                  for ct in range(CT)]
            npair = nt // 2
            pts = [None] * npair
            # two interleaved DVE softmax-denominator accumulation chains
            dnA = dnp.tile([128, CHUNK], F32, name=f"dnA{ch}", tag="dnA")
            dnB = dnp.tile([128, CHUNK], F32, name=f"dnB{ch}", tag="dnB")

            def o_pair(pp):
                for ct in range(CT):
                    nc.tensor.matmul(po[ct][:],
                                     lhsT=vT2[pp][:, :, ct * 128:(ct + 1) * 128],
                                     rhs=pts[pp][:],
                                     start=(pp == 0), stop=(pp == npair - 1),
                                     perf_mode=DR)

            for kt in range(nt):
                ps = psS.tile([128, CHUNK], F32, name=f"ps{ch}_{kt}", tag="pa")
                for t in range(CT // 2):
                    nc.tensor.matmul(ps[:],
                                     lhsT=k2[t][:, :, kt * 128:(kt + 1) * 128],
                                     rhs=qs[t][:],
                                     start=(t == 0), stop=(t == CT // 2 - 1),
                                     perf_mode=DR)
                if kt % 2 == 0:
                    pts[kt // 2] = ptp.tile([128, 2, CHUNK], FP8,
                                            name=f"pt{ch}_{kt}", tag="pt")
                pt_half = pts[kt // 2][:, kt % 2, :]
                nc.scalar.activation(pt_half, ps[:], AF.Exp, scale=scale_s)
                if kt % 2 == 1:
                    pp = kt // 2
                    dn = dnA if pp % 2 == 0 else dnB
                    pt0, pt1 = pts[pp][:, 0, :], pts[pp][:, 1, :]
                    if pp < 2:
                        nc.vector.tensor_add(dn[:], pt0, pt1)
                    else:
                        nc.vector.tensor_add(dn[:], dn[:], pt0)
                        nc.vector.tensor_add(dn[:], dn[:], pt1)
                    # O matmuls lag one completed pair (keeps PE off the ACT path)
                    if kt >= 3:
                        o_pair(pp - 1)
                # previous chunk's out-stores: waits long satisfied by now
                if kt == 4 and pending_stores:
                    for ap_out, osb_t in pending_stores:
                        nc.sync.dma_start(ap_out, osb_t[:])
                    pending_stores = []
                # next chunk's q projection fills PE slack from ring backpressure
                if kt == 25 and ch + 1 < nchunk:
                    qs_next = q_proj(ch + 1)
            o_pair(npair - 1)

            # denominator: merge chains, one f32 ones-matmul does the
            # cross-partition reduce AND the broadcast to all 128 partitions
            nc.vector.tensor_add(dnA[:], dnA[:], dnB[:])
            pbc = psS.tile([128, CHUNK], F32, name=f"pbc{ch}", tag="pa")
            nc.tensor.matmul(pbc[:], lhsT=ones128[:], rhs=dnA[:],
                             start=True, stop=True)
            rb = rbp.tile([128, CHUNK], F32, name=f"rb{ch}", tag="rb")
            nc.vector.reciprocal(rb[:], pbc[:])

            # normalized O -> fp8 pairs (normalization fused into the cast)
            ou = [oup.tile([128, 2, CHUNK], FP8, name=f"ou{ch}_{p}", tag="ou")
                  for p in range(CT // 2)]
            for ct in range(CT):
                nc.vector.tensor_mul(ou[ct // 2][:, ct % 2, :], po[ct][:], rb[:])

            # output projection + residual + bias
            for oct in range(CT):
                pz = psS.tile([128, CHUNK], F32, name=f"pz{ch}_{oct}", tag="pa")
                for t in range(CT // 2):
                    nc.tensor.matmul(pz[:],
                                     lhsT=wo_f8[t][:, :, oct * 128:(oct + 1) * 128],
                                     rhs=ou[t][:],
                                     start=(t == 0), stop=(t == CT // 2 - 1),
                                     perf_mode=DR)
                xr = ep.tile([128, CHUNK], F32, name=f"xr{ch}_{oct}", tag="xr")
                nc.sync.dma_start(xr[:], xd.ap()[oct * 128:(oct + 1) * 128, cols])
                osb = ep.tile([128, CHUNK], F32, name=f"osb{ch}_{oct}", tag="osb")
                nc.vector.scalar_tensor_tensor(osb[:], pz[:], bo_comb[:, oct:oct + 1],
                                               xr[:], op0=ALU.add, op1=ALU.add)
                pending_stores.append(
                    (outd.ap()[oct * 128:(oct + 1) * 128, cols], osb))

            if ch + 1 < nchunk:
                qs = qs_next

        for ap_out, osb_t in pending_stores:
            nc.sync.dma_start(ap_out, osb_t[:])

        if repeat > 1:
            loop_cm.__exit__(None, None, None)

    nc.compile()
    return nc


_NC_CACHE = {}


def _get_nc(n_pix):
    if n_pix not in _NC_CACHE:
        _NC_CACHE[n_pix] = build_nc(n_pix)
    return _NC_CACHE[n_pix]


def make_in_maps(x, gn_scale, gn_bias, Wq, bq, Wk, bk, Wv, bv, Wo, bo):
    B, C_, H, W = x.shape
    n_pix = H * W

    def vec(v):
        return np.ascontiguousarray(
            np.asarray(v, np.float32).reshape(CT, 128).T)

    def wT2(w, dt):
        """wT [C, C] -> pair-packed [CT//2, 128, 2, C] (DoubleRow layout)."""
        wt = np.asarray(w, np.float32).T.reshape(CT // 2, 2, 128, C)
        return np.ascontiguousarray(wt.transpose(0, 2, 1, 3).astype(dt))

    # v-bias folds into the output bias: softmax rows sum to 1, so
    # out = x + Wo @ (v_0 P^T / denom) + (bo + Wo @ bv); the GroupNorm-shift
    # part of the v/q biases is folded on-device (see build_nc).
    bo_eff = (np.asarray(bo, np.float64)
              + np.asarray(Wo, np.float64) @ np.asarray(bv, np.float64))
    bf = ml_dtypes.bfloat16
    f8 = ml_dtypes.float8_e4m3
    base = {
        "wqT2": wT2(Wq, bf),
        "wkT2": wT2(Wk, bf),
        "wvT2": wT2(Wv, bf),
        "woT2": wT2(Wo, f8),
        "gn_scale": vec(gn_scale),
        "gn_bias": vec(gn_bias),
        "bq": vec(bq),
        "bo": vec(bo_eff),
    }
    f32 = lambda v: np.ascontiguousarray(np.asarray(v, np.float32))
    return [dict(base, x=f32(np.asarray(x[b], np.float32).reshape(C_, n_pix)))
            for b in range(B)]


def kernel(x, gn_scale, gn_bias, Wq, bq, Wk, bk, Wv, bv, Wo, bo):
    x = np.asarray(x)
    B, C_, H, W = x.shape
    n_pix = H * W
    nc = _get_nc(n_pix)
    in_maps = make_in_maps(x, gn_scale, gn_bias, Wq, bq, Wk, bk, Wv, bv, Wo, bo)
    res = run_bass_kernel_spmd(nc, in_maps, core_ids=list(range(B)))
    out = np.stack([res.results[b]["out"] for b in range(B)])
    return out.reshape(B, C_, H, W).astype(np.float32)
